# revision 38
# baseline (speedup 1.0000x reference)
"""ResNet BasicBlock forward on 8 Trainium2 NeuronCores.

Computes relu(bn2(conv2(relu(bn1(conv1(x))))) + x) for x[64,128,56,56],
two 3x3 stride-1 pad-1 convs with 128->128 channels, eval-mode BN.

Strategy:
  - Data parallel over batch: 8 images per core, no collectives.
  - Each 3x3 conv = 9 shifted matmuls accumulated in PSUM. Input channels
    (128) sit on the SBUF partition dim (= matmul contraction dim); output
    channels land on PSUM partitions. Spatial output is tiled into 7 PSUM
    banks of 8 rows x 56 cols (448 fp32 = one 2KB bank).
  - bf16 matmul inputs (1 cycle/row on the PE vs 4 for fp32), fp32 PSUM
    accumulation. x is cast to bf16 on the host and DMAd straight into a
    58x58 zero-bordered SBUF buffer, so every shifted 3x3 tap is a plain
    strided access pattern and padding costs nothing. BN scale is folded
    into the conv weights on the host; BN shift is a per-channel bias.
  - Epilogues: ScalarE does relu(psum + b1) -> bf16 mid (padded); VectorE
    does (psum + b2) + residual then relu. The residual is read from the
    bf16 input buffer (adds <0.1% error, saves a 12.8MB/core fp32 stream).
  - Alignment: the kx=1 taps would read the padded buffers at odd element
    offsets (2-byte-misaligned SBUF streams, +7ns per matmul); column-
    shifted twins of x_pad/mid_pad (built by a second input DMA / a second
    epilogue activation) keep every tap's rhs 4B-aligned.
  - Inputs are pre-padded on the host (zero borders baked in) so every
    input DMA is a full-rate contiguous transfer — no on-chip staging,
    scatter, or x-border memsets.
  - Ramp/tail: the ramp is bound by per-DMA fixed latencies (~10.5us to
    first usable chunk: w1 then 3 row-chunks of image 0 on the sync ring,
    image 0's conv1 leading with bank 0 solo); a dummy activation hoists
    the ACT table load off the critical path; 8 warmup matmuls keep the
    HAM clock gate warm across the DMA wait; outputs stream per group;
    the last image's conv2 ends with two 4-row tiles in their own groups
    so the final epilogue+DMA tail is halved. The Tile framework inserts
    all semaphores; images are software-pipelined DEPTH deep.
"""

import functools
import os
import sys

import numpy as np

for _p in ("/opt/trn_rl_repo", "/root/.axon_site/_ro/trn_rl_repo"):
    if os.path.isdir(_p) and _p not in sys.path:
        sys.path.append(_p)

import ml_dtypes  # noqa: E402

import concourse.bass as bass  # noqa: E402,F401
import concourse.mybir as mybir  # noqa: E402
import concourse.tile as tile  # noqa: E402
from concourse import bacc, bass_utils  # noqa: E402

N_CORES = 8
IMGS = 8  # images per core
C = 128
H = W = 56
HP = WP = 58  # padded spatial
RPB = 8  # output rows per PSUM bank
BANKS = H // RPB  # 7
KK = 9  # 3x3 taps
EPS = 1e-5
DEPTH = 4  # image pipeline depth

BF16 = mybir.dt.bfloat16
F32 = mybir.dt.float32


def _build_module():
    nc = bacc.Bacc(
        "TRN2",
        target_bir_lowering=False,
        debug=False,
        enable_asserts=False,
        num_devices=N_CORES,
        enable_partition_id=False,
    )
    # Host-pre-padded input (zero borders included) and its column-shifted
    # twin: every input DMA is then a full-rate contiguous transfer
    # (6.5-6.7KB per partition) instead of a 112-byte-row scatter, and no
    # on-chip staging/scatter or border memsets are needed for x.
    xpad_d = nc.dram_tensor(
        "xpad", [IMGS, C, HP, WP], BF16, kind="ExternalInput"
    ).ap()
    xshd_d = nc.dram_tensor(
        "xshd", [IMGS, C, HP, W], BF16, kind="ExternalInput"
    ).ap()
    w1_d = nc.dram_tensor("w1t", [C, KK, C], BF16, kind="ExternalInput").ap()
    w2_d = nc.dram_tensor("w2t", [C, KK, C], BF16, kind="ExternalInput").ap()
    b1_d = nc.dram_tensor("b1", [C, 1], F32, kind="ExternalInput").ap()
    b2_d = nc.dram_tensor("b2", [C, 1], F32, kind="ExternalInput").ap()
    out_d = nc.dram_tensor("out", [IMGS, C, H, W], F32, kind="ExternalOutput").ap()

    add = mybir.AluOpType.add
    relu = mybir.ActivationFunctionType.Relu

    with tile.TileContext(nc) as tc:
        with (
            tc.tile_pool(name="singles", bufs=1) as singles,
            tc.tile_pool(name="psum", bufs=8, space="PSUM") as psum_pool,
        ):
            w1_sb = singles.tile([C, KK, C], BF16, name="w1_sb")
            w2_sb = singles.tile([C, KK, C], BF16, name="w2_sb")
            b1_sb = singles.tile([C, 1], F32, name="b1_sb")
            b2_sb = singles.tile([C, 1], F32, name="b2_sb")
            dummy = singles.tile([C, 1], F32, name="dummy")
            warm = singles.tile([C, 448], BF16, name="warm")

            nc.vector.memset(dummy, 0.0)
            nc.vector.memset(warm, 0.0)

            x_pad = [
                singles.tile([C, HP, WP], BF16, name=f"x_pad{d}") for d in range(DEPTH)
            ]
            mid_pad = [
                singles.tile([C, HP, WP], BF16, name=f"mid_pad{d}")
                for d in range(DEPTH)
            ]
            out_sb = [
                singles.tile([C, H, W], F32, name=f"out_sb{d}") for d in range(DEPTH)
            ]
            # Column-shifted twins of x_pad/mid_pad: buf[c,y,x] = pad[c,y,x+1].
            # The kx=1 tap's rhs otherwise starts at an odd element offset
            # (2-byte-misaligned SBUF reads) and each such matmul measures
            # +7ns; reading the shifted twin at offset 0 keeps every tap
            # 4B-aligned. Row stride 56*2=112B keeps rows aligned too.
            x_sh = [
                singles.tile([C, HP, W], BF16, name=f"x_sh{d}") for d in range(DEPTH)
            ]
            mid_sh = [
                singles.tile([C, HP, W], BF16, name=f"mid_sh{d}")
                for d in range(DEPTH)
            ]

            def zero_borders(buf):
                nc.gpsimd.memset(buf[:, 0, :], 0.0)
                nc.gpsimd.memset(buf[:, HP - 1, :], 0.0)
                nc.gpsimd.memset(buf[:, 1 : HP - 1, 0 : WP : WP - 1], 0.0)

            def zero_sh_borders(buf):
                # Only rows 0 and HP-1 of the shifted twins are ever read
                # outside the per-image rewrite (cols 0..W-1, all rows).
                nc.gpsimd.memset(buf[:, 0, :], 0.0)
                nc.gpsimd.memset(buf[:, HP - 1, :], 0.0)

            # Borders image 0 needs (x borders arrive pre-zeroed via DMA).
            zero_borders(mid_pad[0])
            zero_sh_borders(mid_sh[0])

            # The ramp is pinned by DMA fixed costs (~3.4us one-time DGE
            # init + ~2.3us pre-transfer + ~3us completion-semaphore
            # latency per dma_start, pipelined along the sync ring): w1
            # first, then image 0 pre-padded in 3 row-chunks landing
            # directly in x_pad (no staging/scatter), so the first chunk's
            # completion gates the first real matmuls as early as the ring
            # allows.
            nc.sync.dma_start(out=w1_sb, in_=w1_d)
            # First chunk = exactly the rows image 0's solo bank-0 group
            # needs: its receipt is the ramp gate, and the chain cost is
            # fixed latencies + transfer, so the smallest useful first
            # chunk wins.
            # Chunk 2 covers exactly group 2's rows: its receipt was
            # measured landing ~0.25us after those matmuls wanted it.
            # Group 3 runs ~1.5us later, so the bigger chunk 3 has slack.
            for r0, r1 in ((0, 10), (10, 28), (28, HP)):
                nc.sync.dma_start(
                    out=x_pad[0][:, r0:r1, :], in_=xpad_d[0][:, r0:r1, :]
                )

            # Hoist the ACT table load off the critical path: the first
            # ACTIVATE in the Scalar stream triggers it.
            nc.scalar.activation(out=dummy, in_=dummy, func=relu)

            # Warm up the PE's HAM clock gate while image 0's DMA is in
            # flight. The first real matmuls are gated by that DMA (~10.3us
            # with the minimal first chunk) while the tensor sequencer
            # starts at ~7.4-8.4us (both run-variable), and HAM
            # re-throttles after ~1.3us of post-warmup idle (costing
            # ~2.2us of half-clock matmuls). 7 warmups end at ~10.0-11.0us
            # (2.4GHz): worst-case stall ~0.3us (stays warm), worst-case
            # overshoot ~0.7us — the best expected value in both clock
            # states now that the gate moved earlier.
            wps = psum_pool.tile([C, 448], F32, name="ps")
            for wi in range(7):
                nc.tensor.matmul(
                    wps,
                    lhsT=warm[:, 0:C],
                    rhs=warm[:, :],
                    start=(wi == 0),
                    stop=(wi == 6),
                )

            # Biases + w2 stay on the sync ring BEHIND the image-0 chunks:
            # SWDGE rides the same 16 SDMA engines, so putting these on
            # the gpsimd ring made them compete with the ramp-critical
            # chunk transfers (measured +1.5us to the first real matmul).
            nc.sync.dma_start(out=b1_sb, in_=b1_d)
            nc.sync.dma_start(out=b2_sb, in_=b2_d)
            nc.sync.dma_start(out=w2_sb, in_=w2_d)

            # Zero the remaining mid buffers' borders (GpSimd — nothing
            # else runs there). x buffers need none: their DMAs write the
            # full pre-padded tile. Interiors are fully rewritten per
            # image.
            for buf in mid_pad[1:]:
                zero_borders(buf)
            for buf in mid_sh[1:]:
                zero_sh_borders(buf)

            for i in range(IMGS):
                d = i % DEPTH
                xp, mp, ob = x_pad[d], mid_pad[d], out_sb[d]
                xs, ms = x_sh[d], mid_sh[d]
                if i > 0:
                    nc.sync.dma_start(out=xp, in_=xpad_d[i])
                    nc.sync.dma_start(out=xs, in_=xshd_d[i])

                # Output rows are tiled into groups of row-ranges; tiles
                # in a group share each tap's weight load (consecutive
                # matmuls with the same stationary operand overlap it).
                # Image 0's conv1 leads with bank 0 solo so its first
                # matmuls gate only on the small first input chunk. The
                # last image's conv2 ends with two 4-row tiles in their
                # own groups: the penultimate epilogue hides under the
                # final tile's matmuls, halving the post-last-matmul tail.
                pairs = [
                    ((0, 8), (8, 16)),
                    ((16, 24), (24, 32)),
                    ((32, 40), (40, 48)),
                    ((48, 56),),
                ]
                groups1 = (
                    [
                        ((0, 8),),
                        ((8, 16), (16, 24)),
                        ((24, 32), (32, 40)),
                        ((40, 48), (48, 56)),
                    ]
                    if i == 0
                    else pairs
                )
                groups2 = (
                    pairs[:3] + [((48, 52),), ((52, 56),)]
                    if i == IMGS - 1
                    else pairs
                )

                # conv1 + bn1 + relu -> mid (bf16, padded) + shifted twin
                for group in groups1:
                    pts = [
                        psum_pool.tile([C, r1 - r0, W], F32, name="ps")
                        for r0, r1 in group
                    ]
                    for kk in range(KK):
                        ky, kx = divmod(kk, 3)
                        for ps, (r0, r1) in zip(pts, group):
                            # kx=1 reads the aligned shifted twin (image 0
                            # has no twin staged; its 21 misaligned MMs
                            # cost ~150ns total, cheaper than a third
                            # ramp-critical input DMA).
                            if kx == 1 and i > 0:
                                rhs = xs[:, r0 + ky : r1 + ky, 0:W]
                            else:
                                rhs = xp[:, r0 + ky : r1 + ky, kx : kx + W]
                            nc.tensor.matmul(
                                ps,
                                lhsT=w1_sb[:, kk, :],
                                rhs=rhs,
                                start=(kk == 0),
                                stop=(kk == KK - 1),
                            )
                    for ps, (r0, r1) in zip(pts, group):
                        nc.scalar.activation(
                            out=mp[:, 1 + r0 : 1 + r1, 1 : W + 1],
                            in_=ps,
                            func=relu,
                            bias=b1_sb[:, 0:1],
                        )
                        # Second write into the shifted twin keeps conv2's
                        # kx=1 taps 4B-aligned too. ACT has the slack and
                        # each group's drain window fits both.
                        nc.scalar.activation(
                            out=ms[:, 1 + r0 : 1 + r1, 0:W],
                            in_=ps,
                            func=relu,
                            bias=b1_sb[:, 0:1],
                        )

                # conv2 + bn2 + residual + relu -> out
                for group in groups2:
                    pts2 = [
                        psum_pool.tile([C, r1 - r0, W], F32, name="ps")
                        for r0, r1 in group
                    ]
                    for kk in range(KK):
                        ky, kx = divmod(kk, 3)
                        for ps2, (r0, r1) in zip(pts2, group):
                            if kx == 1:
                                rhs = ms[:, r0 + ky : r1 + ky, 0:W]
                            else:
                                rhs = mp[:, r0 + ky : r1 + ky, kx : kx + W]
                            nc.tensor.matmul(
                                ps2,
                                lhsT=w2_sb[:, kk, :],
                                rhs=rhs,
                                start=(kk == 0),
                                stop=(kk == KK - 1),
                            )
                    for ps2, (r0, r1) in zip(pts2, group):
                        rows = ob[:, r0:r1, :]
                        nc.vector.scalar_tensor_tensor(
                            out=rows,
                            in0=ps2,
                            scalar=b2_sb[:, 0:1],
                            in1=xp[:, 1 + r0 : 1 + r1, 1 : W + 1],
                            op0=add,
                            op1=add,
                        )
                        if i == IMGS - 1:
                            # ScalarE takes the final relus: the last
                            # image's drain is otherwise a serial DVE
                            # chain past the last matmul, and ACT is idle
                            # once its conv1 writes are done.
                            nc.scalar.activation(out=rows, in_=rows, func=relu)
                            nc.scalar.dma_start(
                                out=out_d[i][:, r0:r1, :], in_=rows
                            )
                        else:
                            nc.vector.tensor_scalar_max(rows, rows, 0.0)
                    # Stream the output out per group instead of one 1.6MB
                    # DMA per image: spreads the SBUF-read burst across
                    # the conv instead of landing it all at the next
                    # conv's start.
                    if i < IMGS - 1:
                        lo, hi = group[0][0], group[-1][1]
                        nc.scalar.dma_start(
                            out=out_d[i][:, lo:hi, :], in_=ob[:, lo:hi, :]
                        )

    nc.compile()
    return nc


def _install_neff_cache():
    """Content-addressed on-disk cache for walrus NEFF compiles.

    The BIR JSON for this module is byte-identical across processes, so a
    fresh process can reuse the NEFF compiled by an earlier one instead of
    paying the multi-minute walrus compile again.
    """
    import hashlib
    import shutil

    from concourse import bass2jax, bass_utils as bu

    if getattr(bu, "_neff_cache_installed", False):
        return
    orig = bu.compile_bir_kernel
    cache_dir = "/var/tmp/bass_neff_cache"

    def cached(bir_json, tmpdir, neff_name="file.neff"):
        data = bir_json if isinstance(bir_json, bytes) else bir_json.encode()
        key = hashlib.sha256(data).hexdigest()
        cpath = os.path.join(cache_dir, key + ".neff")
        try:
            if os.path.exists(cpath):
                dst = os.path.join(tmpdir, neff_name)
                shutil.copy(cpath, dst)
                return dst
        except OSError:
            pass
        neff_path = orig(bir_json, tmpdir, neff_name)
        try:
            os.makedirs(cache_dir, exist_ok=True)
            tmp = cpath + f".tmp{os.getpid()}"
            shutil.copy(neff_path, tmp)
            os.replace(tmp, cpath)
        except OSError:
            pass
        return neff_path

    bu.compile_bir_kernel = cached
    bass2jax.compile_bir_kernel = cached
    bu._neff_cache_installed = True


@functools.lru_cache(maxsize=1)
def _get_module():
    _install_neff_cache()
    return _build_module()


def _prep_in_maps(inputs):
    f32 = np.float32
    x = np.asarray(inputs["x"], f32)
    w1 = np.asarray(inputs["w1"], f32)
    w2 = np.asarray(inputs["w2"], f32)
    gamma1 = np.asarray(inputs["gamma1"], f32)
    beta1 = np.asarray(inputs["beta1"], f32)
    mean1 = np.asarray(inputs["mean1"], f32)
    var1 = np.asarray(inputs["var1"], f32)
    gamma2 = np.asarray(inputs["gamma2"], f32)
    beta2 = np.asarray(inputs["beta2"], f32)
    mean2 = np.asarray(inputs["mean2"], f32)
    var2 = np.asarray(inputs["var2"], f32)

    a1 = gamma1 / np.sqrt(var1 + EPS)
    a2 = gamma2 / np.sqrt(var2 + EPS)
    # Fold BN scale into weights; transpose to [c_in, tap, c_out] for lhsT.
    w1t = np.ascontiguousarray(
        np.transpose(w1 * a1[:, None, None, None], (1, 2, 3, 0)).reshape(C, KK, C)
    ).astype(ml_dtypes.bfloat16)
    w2t = np.ascontiguousarray(
        np.transpose(w2 * a2[:, None, None, None], (1, 2, 3, 0)).reshape(C, KK, C)
    ).astype(ml_dtypes.bfloat16)
    b1 = np.ascontiguousarray((beta1 - mean1 * a1).reshape(C, 1).astype(f32))
    b2 = np.ascontiguousarray((beta2 - mean2 * a2).reshape(C, 1).astype(f32))

    # Pre-pad on the host (zero borders baked in) and build the column-
    # shifted twin; both land as fully contiguous per-partition DMAs.
    xbf = np.ascontiguousarray(x).astype(ml_dtypes.bfloat16)
    xpad = np.zeros((x.shape[0], C, HP, WP), ml_dtypes.bfloat16)
    xpad[:, :, 1 : H + 1, 1 : W + 1] = xbf
    xshd = np.ascontiguousarray(xpad[:, :, :, 1 : W + 1])
    return [
        {
            "xpad": xpad[IMGS * i : IMGS * (i + 1)],
            "xshd": xshd[IMGS * i : IMGS * (i + 1)],
            "w1t": w1t,
            "w2t": w2t,
            "b1": b1,
            "b2": b2,
        }
        for i in range(N_CORES)
    ]


def _run(inputs, trace=False):
    nc = _get_module()
    in_maps = _prep_in_maps(inputs)
    res = bass_utils.run_bass_kernel_spmd(
        nc, in_maps, core_ids=list(range(N_CORES)), trace=trace
    )
    out = np.concatenate([r["out"] for r in res.results], axis=0)
    return out.astype(np.float32), res


def kernel(**inputs):
    out, _ = _run(inputs, trace=False)
    return out

